# revision 37
# baseline (speedup 1.0000x reference)
"""Trainium2 Bass kernel for the supervoxel erode/edge loss module.

Math: the reference pads the [B,X,Y] grid by (4sx, 4sy), tiles it into 8x8
patches, zeroes each patch's last row/col of the mask channel, erodes along
both patch axes (`a*b + (1-a)*a + (1-b)*a` == `2a - a^2` with a the
neighbor product) and sums eroded*edge over all cells, then takes the mean
over (B, patches).  In padded coords u = x + 4sx, v = y + 4sy the patch
grid is [0,1032)^2; a cell contributes iff u%8 <= 5 and v%8 <= 5, and its
contribution uses only raw mask values:

    ax = m(u,v)*m(u+1,v); ay = m(u,v)*m(u,v+1)
    term = ax*(2-ax) * ay*(2-ay) * e(u,v)

so the loss decomposes into independent 7-row blocks (u in [8k, 8k+6]) x
7-col groups (v in [8g, 8g+6]) with a 6x6 live base grid per block/group.
With Q(a) = (a-1)^2 (so a(2-a) = 1-Q(a)):

    term = (1-Qx)(1-Qy) e = z - w,   w = (Qy-1) e,   z = Qx w
    total = sum(z) - sum(w)

Host staging (pure slicing + zero-fill, no arithmetic on values), bf16:
    mask chunks [128, 7*wg*7] -- per (block-tile, column-chunk); the first
                tile is split in quarters so the first DMA lands early
    edge halves [128, 6*hg*6] -- per (block-tile, column-half)
    runtb [128, 4*W] -- leftover (<128) blocks' vb/vr/vc/e cells gathered
                dense across all 128 partitions
Cores take contiguous block ranges (data-parallel; the mean is one scalar
so the combine happens on host, no collective).

Device, per subunit (tile x column-chunk; p = block), 4-engine pipeline:

    A (DVE):  ay0 = vb*vc ; ax0 = vb*vr          (bf16 muls run 2x)
    B (Act):  sqy = Square(ay0 - 1) ; sqx = Square(ax0 - 1)
    C (DVE):  w = (sqy-1)*e  [fused accum Sw] ; z = sqx*w (Pool option)
    PE     :  zacc(PSUM) += I @ z   (z-sum accumulation on the idle PE)

and one final Act Copy-accum drains zacc into the Sz partial.  DVE is the
critical engine; Act/PE ride under it.  (Custom DVE ops and native
tensor_tensor_reduce would fuse more but crash this runtime; heavier
Pool/Act offload thrashes shared SBUF bandwidth.)

DMA: everything up front on the gpsimd software-DGE queue (hardware-DGE
queues cap at ~25-37 GB/s; SWDGE sustains ~310-344 GB/s) as one
3-12 KiB descriptor per (block, input, chunk) -- packet cadence, not
bandwidth, is the real limit, so descriptors stay big and few (~1400/core
vs the naive layout's ~5500 packets).
"""

import sys

sys.path.insert(0, "/opt/trn_rl_repo")

import numpy as np

from concourse import bacc, mybir, tile
from concourse.bass_utils import run_bass_kernel_spmd

F32 = mybir.dt.float32
BF16 = mybir.dt.bfloat16
N_CORES = 8
SHIFTS = [(0, 0), (1, 0), (0, 1), (1, 1)]
DX = 8


def _chunks_for(NG, n_full):
    """Column-chunk plan: first tile in quarters (early first DMA), last
    tile ends in quarters (short pipeline tail), middle tiles in halves."""
    h = NG // 2
    q = NG // 4
    plans = []
    for u in range(n_full):
        if u == 0 and q >= 8:
            plans.append([(0, q), (q, h), (h, h + q), (h + q, NG)])
        elif u == n_full - 1 and NG // 8 >= 8:
            e = NG // 8
            plans.append([(0, h), (h, h + q), (h + q, h + q + e), (h + q + e, NG)])
        else:
            plans.append([(0, h), (h, NG)])
    return plans


def _build_program(n_full, NG, W_runt, niter=1, pool_z=False):
    """Per-core program. Inputs (bf16): mask chunks m{k} [128, 7*wg*7],
    edge halves e{k} [128, 6*hg*6], runtb [128, 4*W]. Output: out [128, U]
    f32: per-subunit Sw columns, then one total Sz column."""
    h = NG // 2
    plans = _chunks_for(NG, n_full)
    subs = [(u, ci) for u in range(n_full) for ci in range(len(plans[u]))]
    subs += [(-1, 0)] if W_runt else []
    NSU = len(subs)
    # last two subunits sum z directly (STT accum) so the PSUM drain's
    # dependencies complete early and it hides under the pipeline tail
    direct_z = {i for i in (NSU - 1, NSU - 2) if i >= 0}
    U = NSU + 1 + len(direct_z)

    nc = bacc.Bacc("TRN2", target_bir_lowering=False, debug=False)
    mh = {}
    for u in range(n_full):
        for ci, (g0, g1) in enumerate(plans[u]):
            mh[(u, ci)] = nc.dram_tensor(
                f"m{u}_{ci}", [128, 7 * (g1 - g0) * 7], BF16, kind="ExternalInput"
            )
    eh = {}
    for u in range(n_full):
        for s, (g0, g1) in enumerate([(0, h), (h, NG)]):
            eh[(u, s)] = nc.dram_tensor(
                f"e{u}_{s}", [128, 6 * (g1 - g0) * 6], BF16, kind="ExternalInput"
            )
    if W_runt:
        runtb_h = nc.dram_tensor("runtb", [128, 4 * W_runt], BF16, kind="ExternalInput")
    ident_h = nc.dram_tensor("ident", [128, 128], BF16, kind="ExternalInput")
    out_h = nc.dram_tensor("out", [128, U], F32, kind="ExternalOutput")

    with tile.TileContext(nc) as tc:
        with (
            tc.tile_pool(name="mt", bufs=sum(len(p) for p in plans)) as mt_pool,
            tc.tile_pool(name="et", bufs=2 * n_full) as et_pool,
            tc.tile_pool(name="wa", bufs=3) as wa_pool,
            tc.tile_pool(name="ws", bufs=3) as ws_pool,
            tc.tile_pool(name="wz", bufs=2) as wz_pool,
            tc.tile_pool(name="ps", bufs=1, space="PSUM") as ps_pool,
            tc.tile_pool(name="c", bufs=1) as c_pool,
        ):
            partials = c_pool.tile([128, U], F32)
            bm1 = c_pool.tile([128, 1], F32, name="bm1")
            ident = c_pool.tile([128, 128], BF16, name="ident")
            WACC = 6 * (NG - h) * 6
            zjunk = c_pool.tile([128, WACC], BF16, name="zjunk")
            zacc = ps_pool.tile([128, WACC], F32, name="zacc")
            rt = c_pool.tile([128, 4 * W_runt], BF16, name="rt") if W_runt else None

            def emit_iter():
                nc.vector.memset(bm1[:], -1.0)
                nc.sync.dma_start(ident[:], ident_h.ap())
                # DMA queue order: per tile, mask chunks then edge halves;
                # the tiny runt rides last (its compute is last too)
                mtiles, etiles = {}, {}
                for u in range(n_full):
                    for ci, (g0, g1) in enumerate(plans[u]):
                        mt = mt_pool.tile([128, 7 * (g1 - g0) * 7], BF16, name="mt")
                        nc.gpsimd.dma_start(mt[:], mh[(u, ci)].ap())
                        mtiles[(u, ci)] = mt
                    for s, (g0, g1) in enumerate([(0, h), (h, NG)]):
                        et = et_pool.tile([128, 6 * (g1 - g0) * 6], BF16, name="et")
                        nc.gpsimd.dma_start(et[:], eh[(u, s)].ap())
                        etiles[(u, s)] = et
                if W_runt:
                    nc.gpsimd.dma_start(rt[:], runtb_h.ap())

                sq = mybir.ActivationFunctionType.Square
                st, zs = {}, {}

                def sub_views(i):
                    u, ci = subs[i]
                    if u >= 0:
                        g0, g1 = plans[u][ci]
                        wg = g1 - g0
                        mt = mtiles[(u, ci)]
                        m4 = mt[:].rearrange("p (r g j) -> p r g j", r=7, j=7)
                        s = 0 if g1 <= h else 1
                        e0 = 0 if s == 0 else h
                        et = etiles[(u, s)]
                        e4 = et[:].rearrange(
                            "p (r g j) -> p r g j", r=6, j=6
                        )[:, :, g0 - e0 : g1 - e0, :]
                        n = 6 * wg * 6
                        return (
                            m4[:, 0:6, :, 0:6], m4[:, 1:7, :, 0:6],
                            m4[:, 0:6, :, 1:7], e4, n, True,
                        )
                    W = W_runt
                    return (
                        rt[:, 0:W], rt[:, W : 2 * W], rt[:, 2 * W : 3 * W],
                        rt[:, 3 * W : 4 * W], W, False,
                    )

                def swv(t, full):
                    return (t[:].rearrange("p (r g j) -> p r g j", r=6, j=6)
                            if full else t[:])

                def stage_a(i):
                    vb, vr, vc, ev, n, full = sub_views(i)
                    ay0 = wa_pool.tile([128, n], BF16, name="ay0")
                    ax0 = wa_pool.tile([128, n], BF16, name="ax0")
                    nc.vector.tensor_mul(swv(ay0, full), vb, vc)
                    nc.vector.tensor_mul(swv(ax0, full), vb, vr)
                    st[i] = (ax0, ay0, ev, n)

                def stage_b(i):
                    ax0, ay0, ev, n = st[i]
                    sqy = ws_pool.tile([128, n], BF16, name="sqy")
                    sqx = ws_pool.tile([128, n], BF16, name="sqx")
                    nc.scalar.activation(sqy[:], ay0[:], sq, bias=bm1[:])
                    nc.scalar.activation(sqx[:], ax0[:], sq, bias=bm1[:])
                    st[i] = (sqx, sqy, ev, n)

                zcol = {}
                for k, i in enumerate(sorted(direct_z)):
                    zcol[i] = NSU + 1 + k

                def stage_c(i):
                    sqx, sqy, ev, n = st.pop(i)
                    w = wz_pool.tile([128, n], BF16, name="w")
                    z = wz_pool.tile([128, n], BF16, name="z")
                    nc.vector.scalar_tensor_tensor(
                        w[:], sqy[:], 1.0, ev,
                        op0=mybir.AluOpType.subtract, op1=mybir.AluOpType.mult,
                        accum_out=partials[:, i : i + 1],
                    )
                    if i in direct_z:
                        nc.vector.scalar_tensor_tensor(
                            z[:], sqx[:], 0.0, w[:],
                            op0=mybir.AluOpType.add, op1=mybir.AluOpType.mult,
                            accum_out=partials[:, zcol[i] : zcol[i] + 1],
                        )
                    else:
                        zmul = (nc.gpsimd.tensor_mul if pool_z
                                else nc.vector.tensor_mul)
                        zmul(z[:], sqx[:], w[:])
                        zs[i] = (z, n)

                # PSUM range bookkeeping: first/last subunit touching each
                sub_n = []
                for i in range(NSU):
                    u, ci = subs[i]
                    sub_n.append(
                        6 * (plans[u][ci][1] - plans[u][ci][0]) * 6
                        if u >= 0 else W_runt
                    )
                r_first, r_last = {}, {}
                for i, n in enumerate(sub_n):
                    if i in direct_z:
                        continue
                    for c0 in range(0, n, 512):
                        r = c0 // 512
                        r_first.setdefault(r, i)
                        r_last[r] = i

                def stage_pe(i):
                    if i in direct_z:
                        return
                    z, n = zs.pop(i)
                    for c0 in range(0, n, 512):
                        cw = min(512, n - c0)
                        r = c0 // 512
                        nc.tensor.matmul(
                            zacc[:, c0 : c0 + cw], ident[:], z[:, c0 : c0 + cw],
                            start=(r_first[r] == i), stop=(r_last[r] == i),
                        )

                stage_a(0)
                stage_b(0)
                for i in range(1, NSU):
                    stage_a(i)
                    stage_b(i)
                    stage_c(i - 1)
                    stage_pe(i - 1)
                stage_c(NSU - 1)
                stage_pe(NSU - 1)
                nc.scalar.activation(
                    zjunk[:], zacc[:], mybir.ActivationFunctionType.Copy,
                    accum_out=partials[:, NSU : NSU + 1],
                )
                nc.sync.dma_start(out_h.ap(), partials[:])

            if niter == 1:
                emit_iter()
            else:
                with tc.For_i(0, niter, 1):
                    emit_iter()

    nc.compile()
    return nc


def _stage(mask, edge, idx):
    """Host-side slicing: maskb [B*KXb, 7, NG, 7] and edgeb [B*KXb, 6, NG, 6]
    f32 (dense; mask zero outside the image)."""
    B, X, Y = mask.shape
    sx, sy = SHIFTS[idx]
    KX = (X + DX) // DX
    KY = (Y + DX) // DX

    gy = np.arange(KY)
    y0 = 8 * gy[:, None] + np.arange(7)[None, :] - 4 * sy  # [KY, 7]
    g_ok = (y0 >= 0) & (y0 < Y)
    gsel = np.nonzero(g_ok.any(axis=1))[0]
    NG = len(gsel)
    ym = y0[gsel]
    yv = g_ok[gsel]
    kxs = np.arange(KX)
    x0 = 8 * kxs[:, None] + np.arange(7)[None, :] - 4 * sx  # [KX, 7]
    k_ok = (x0 >= 0) & (x0 < X)
    ksel = np.nonzero(k_ok.any(axis=1))[0]
    KXb = len(ksel)
    xm = x0[ksel]
    xv = k_ok[ksel]

    mc = mask[:, np.clip(xm.ravel(), 0, X - 1), :][:, :, np.clip(ym.ravel(), 0, Y - 1)]
    mc = mc.reshape(B, KXb * 7, NG * 7)
    vmask = (xv.ravel()[:, None] & yv.ravel()[None, :]).astype(mask.dtype)
    mc *= vmask
    maskb = mc.reshape(B * KXb, 7, NG, 7)

    xe = np.clip(xm[:, 0:6].ravel(), 0, X - 1)
    ye = np.clip(ym[:, 0:6].ravel(), 0, Y - 1)
    edgeb = edge[:, xe, :][:, :, ye].reshape(B * KXb, 6, NG, 6)

    norm = B * KX * KY
    return maskb, edgeb, NG, KXb, norm


def _stage_runt(maskb4, edgeb4, sel):
    """Gather leftover blocks' vb/vr/vc/e cells into [128, 4*W] (W padded)."""
    m4 = maskb4[sel]
    e4 = edgeb4[sel]
    vb = m4[:, 0:6, :, 0:6].ravel()
    vr = m4[:, 1:7, :, 0:6].ravel()
    vc = m4[:, 0:6, :, 1:7].ravel()
    ee = e4.ravel()
    n = vb.size
    W = -(-n // 128)
    out = np.zeros((4, 128 * W), dtype=maskb4.dtype)
    for i, a in enumerate((vb, vr, vc, ee)):
        out[i, :n] = a
    return np.ascontiguousarray(
        out.reshape(4, 128, W).transpose(1, 0, 2).reshape(128, 4 * W)
    ), W


def _run(mask, edge, loss_old, idx, trace=False, niter=1, pool_z=False):
    import ml_dtypes

    B, X, Y, _ = mask.shape
    assert B % N_CORES == 0
    m3 = np.ascontiguousarray(mask[..., idx], dtype=np.float32)
    e3 = np.ascontiguousarray(edge[..., 0], dtype=np.float32)
    maskb, edgeb, NG, KXb, norm = _stage(m3, e3, idx)

    NBtot = maskb.shape[0]
    assert NBtot % N_CORES == 0
    NBc = NBtot // N_CORES
    n_full = NBc // 128
    runt_sel0 = np.arange(n_full * 128, NBc)
    h = NG // 2

    maskb4 = maskb.astype(ml_dtypes.bfloat16)
    edgeb4 = edgeb.astype(ml_dtypes.bfloat16)
    plans = _chunks_for(NG, n_full)
    ident = np.eye(128, dtype=ml_dtypes.bfloat16)

    in_maps = []
    W_runt = 0
    for i in range(N_CORES):
        lo = i * NBc
        im = {"ident": ident}
        for u in range(n_full):
            r0 = lo + u * 128
            for ci, (g0, g1) in enumerate(plans[u]):
                im[f"m{u}_{ci}"] = np.ascontiguousarray(
                    maskb4[r0 : r0 + 128, :, g0:g1, :]
                ).reshape(128, -1)
            for s, (g0, g1) in enumerate([(0, h), (h, NG)]):
                im[f"e{u}_{s}"] = np.ascontiguousarray(
                    edgeb4[r0 : r0 + 128, :, g0:g1, :]
                ).reshape(128, -1)
        if len(runt_sel0):
            rb, W_runt = _stage_runt(maskb4, edgeb4, lo + runt_sel0)
            im["runtb"] = rb
        in_maps.append(im)

    nc = _build_program(n_full, NG, W_runt, niter=niter, pool_z=pool_z)
    res = run_bass_kernel_spmd(nc, in_maps, list(range(N_CORES)), trace=trace)
    # w columns are the first NSU; z columns (zacc total + direct) follow
    NSU = sum(len(p) for p in plans) + (1 if W_runt else 0)
    total = 0.0
    for i in range(N_CORES):
        o = np.asarray(res.results[i]["out"], np.float64)
        total += o[:, NSU:].sum() - o[:, :NSU].sum()
    out = np.float32(np.asarray(loss_old, dtype=np.float32) + total / norm)
    return np.asarray(out, dtype=np.float32), res


def kernel(resized_image, mask_combined, edge_map, loss_old, mask_index):
    mask = np.asarray(mask_combined, dtype=np.float32)
    edge = np.asarray(edge_map, dtype=np.float32)
    idx = int(np.asarray(mask_index))
    out, _ = _run(mask, edge, loss_old, idx)
    return out


# revision 38
# speedup vs baseline: 1.0237x; 1.0237x over previous
"""Trainium2 Bass kernel for the supervoxel erode/edge loss module.

Math: the reference pads the [B,X,Y] grid by (4sx, 4sy), tiles it into 8x8
patches, zeroes each patch's last row/col of the mask channel, erodes along
both patch axes (`a*b + (1-a)*a + (1-b)*a` == `2a - a^2` with a the
neighbor product) and sums eroded*edge over all cells, then takes the mean
over (B, patches).  In padded coords u = x + 4sx, v = y + 4sy the patch
grid is [0,1032)^2; a cell contributes iff u%8 <= 5 and v%8 <= 5, and its
contribution uses only raw mask values:

    ax = m(u,v)*m(u+1,v); ay = m(u,v)*m(u,v+1)
    term = ax*(2-ax) * ay*(2-ay) * e(u,v)

so the loss decomposes into independent 7-row blocks (u in [8k, 8k+6]) x
7-col groups (v in [8g, 8g+6]) with a 6x6 live base grid per block/group.
With Q(a) = (a-1)^2 (so a(2-a) = 1-Q(a)):

    term = (1-Qx)(1-Qy) e = z - w,   w = (Qy-1) e,   z = Qx w
    total = sum(z) - sum(w)

Host staging (pure slicing + zero-fill, no arithmetic on values), bf16:
    mask chunks [128, 7*wg*7] -- per (block-tile, column-chunk); the first
                tile is split in quarters so the first DMA lands early
    edge halves [128, 6*hg*6] -- per (block-tile, column-half)
    runtb [128, 4*W] -- leftover (<128) blocks' vb/vr/vc/e cells gathered
                dense across all 128 partitions
Cores take contiguous block ranges (data-parallel; the mean is one scalar
so the combine happens on host, no collective).

Device, per subunit (tile x column-chunk; p = block), 4-engine pipeline:

    A (DVE):  ay0 = vb*vc ; ax0 = vb*vr          (bf16 muls run 2x)
    B (Act):  sqy = Square(ay0 - 1) ; sqx = Square(ax0 - 1)
    C (DVE):  w = (sqy-1)*e  [fused accum Sw] ; z = sqx*w (Pool option)
    PE     :  zacc(PSUM) += I @ z   (z-sum accumulation on the idle PE)

and one final Act Copy-accum drains zacc into the Sz partial.  DVE is the
critical engine; Act/PE ride under it.  (Custom DVE ops and native
tensor_tensor_reduce would fuse more but crash this runtime; heavier
Pool/Act offload thrashes shared SBUF bandwidth.)

DMA: everything up front on the gpsimd software-DGE queue (hardware-DGE
queues cap at ~25-37 GB/s; SWDGE sustains ~310-344 GB/s) as one
3-12 KiB descriptor per (block, input, chunk) -- packet cadence, not
bandwidth, is the real limit, so descriptors stay big and few (~1400/core
vs the naive layout's ~5500 packets).
"""

import sys

sys.path.insert(0, "/opt/trn_rl_repo")

import numpy as np

from concourse import bacc, mybir, tile
from concourse.bass_utils import run_bass_kernel_spmd

F32 = mybir.dt.float32
BF16 = mybir.dt.bfloat16
N_CORES = 8
SHIFTS = [(0, 0), (1, 0), (0, 1), (1, 1)]
DX = 8


def _chunks_for(NG, n_full):
    """Column-chunk plan: first tile in quarters (early first DMA), last
    tile ends in quarters (short pipeline tail), middle tiles in halves."""
    h = NG // 2
    q = NG // 4
    plans = []
    for u in range(n_full):
        if u == 0 and q >= 8:
            plans.append([(0, q), (q, h), (h, h + q), (h + q, NG)])
        elif u == n_full - 1 and q >= 8:
            plans.append([(0, h), (h, h + q), (h + q, NG)])
        else:
            plans.append([(0, h), (h, NG)])
    return plans


def _build_program(n_full, NG, W_runt, niter=1, pool_z=False):
    """Per-core program. Inputs (bf16): mask chunks m{k} [128, 7*wg*7],
    edge halves e{k} [128, 6*hg*6], runtb [128, 4*W]. Output: out [128, U]
    f32: per-subunit Sw columns, then one total Sz column."""
    h = NG // 2
    plans = _chunks_for(NG, n_full)
    subs = [(u, ci) for u in range(n_full) for ci in range(len(plans[u]))]
    subs += [(-1, 0)] if W_runt else []
    NSU = len(subs)
    # last two subunits sum z directly (STT accum) so the PSUM drain's
    # dependencies complete early and it hides under the pipeline tail
    direct_z = {i for i in (NSU - 1, NSU - 2) if i >= 0}
    U = NSU + 1 + len(direct_z)

    nc = bacc.Bacc("TRN2", target_bir_lowering=False, debug=False)
    mh = {}
    for u in range(n_full):
        for ci, (g0, g1) in enumerate(plans[u]):
            mh[(u, ci)] = nc.dram_tensor(
                f"m{u}_{ci}", [128, 7 * (g1 - g0) * 7], BF16, kind="ExternalInput"
            )
    eh = {}
    for u in range(n_full):
        for s, (g0, g1) in enumerate([(0, h), (h, NG)]):
            eh[(u, s)] = nc.dram_tensor(
                f"e{u}_{s}", [128, 6 * (g1 - g0) * 6], BF16, kind="ExternalInput"
            )
    if W_runt:
        runtb_h = nc.dram_tensor("runtb", [128, 4 * W_runt], BF16, kind="ExternalInput")
    ident_h = nc.dram_tensor("ident", [128, 128], BF16, kind="ExternalInput")
    out_h = nc.dram_tensor("out", [128, U], F32, kind="ExternalOutput")

    with tile.TileContext(nc) as tc:
        with (
            tc.tile_pool(name="mt", bufs=sum(len(p) for p in plans)) as mt_pool,
            tc.tile_pool(name="et", bufs=2 * n_full) as et_pool,
            tc.tile_pool(name="wa", bufs=3) as wa_pool,
            tc.tile_pool(name="ws", bufs=3) as ws_pool,
            tc.tile_pool(name="wz", bufs=2) as wz_pool,
            tc.tile_pool(name="ps", bufs=1, space="PSUM") as ps_pool,
            tc.tile_pool(name="c", bufs=1) as c_pool,
        ):
            partials = c_pool.tile([128, U], F32)
            bm1 = c_pool.tile([128, 1], F32, name="bm1")
            ident = c_pool.tile([128, 128], BF16, name="ident")
            WACC = 6 * (NG - h) * 6
            zjunk = c_pool.tile([128, WACC], BF16, name="zjunk")
            zacc = ps_pool.tile([128, WACC], F32, name="zacc")
            rt = c_pool.tile([128, 4 * W_runt], BF16, name="rt") if W_runt else None

            def emit_iter():
                nc.vector.memset(bm1[:], -1.0)
                nc.sync.dma_start(ident[:], ident_h.ap())
                # DMA queue order: per tile, mask chunks then edge halves;
                # the tiny runt rides last (its compute is last too)
                mtiles, etiles = {}, {}
                for u in range(n_full):
                    for ci, (g0, g1) in enumerate(plans[u]):
                        mt = mt_pool.tile([128, 7 * (g1 - g0) * 7], BF16, name="mt")
                        nc.gpsimd.dma_start(mt[:], mh[(u, ci)].ap())
                        mtiles[(u, ci)] = mt
                    for s, (g0, g1) in enumerate([(0, h), (h, NG)]):
                        et = et_pool.tile([128, 6 * (g1 - g0) * 6], BF16, name="et")
                        nc.gpsimd.dma_start(et[:], eh[(u, s)].ap())
                        etiles[(u, s)] = et
                if W_runt:
                    nc.gpsimd.dma_start(rt[:], runtb_h.ap())

                sq = mybir.ActivationFunctionType.Square
                st, zs = {}, {}

                def sub_views(i):
                    u, ci = subs[i]
                    if u >= 0:
                        g0, g1 = plans[u][ci]
                        wg = g1 - g0
                        mt = mtiles[(u, ci)]
                        m4 = mt[:].rearrange("p (r g j) -> p r g j", r=7, j=7)
                        s = 0 if g1 <= h else 1
                        e0 = 0 if s == 0 else h
                        et = etiles[(u, s)]
                        e4 = et[:].rearrange(
                            "p (r g j) -> p r g j", r=6, j=6
                        )[:, :, g0 - e0 : g1 - e0, :]
                        n = 6 * wg * 6
                        return (
                            m4[:, 0:6, :, 0:6], m4[:, 1:7, :, 0:6],
                            m4[:, 0:6, :, 1:7], e4, n, True,
                        )
                    W = W_runt
                    return (
                        rt[:, 0:W], rt[:, W : 2 * W], rt[:, 2 * W : 3 * W],
                        rt[:, 3 * W : 4 * W], W, False,
                    )

                def swv(t, full):
                    return (t[:].rearrange("p (r g j) -> p r g j", r=6, j=6)
                            if full else t[:])

                def stage_a(i):
                    vb, vr, vc, ev, n, full = sub_views(i)
                    ay0 = wa_pool.tile([128, n], BF16, name="ay0")
                    ax0 = wa_pool.tile([128, n], BF16, name="ax0")
                    nc.vector.tensor_mul(swv(ay0, full), vb, vc)
                    nc.vector.tensor_mul(swv(ax0, full), vb, vr)
                    st[i] = (ax0, ay0, ev, n)

                def stage_b(i):
                    ax0, ay0, ev, n = st[i]
                    sqy = ws_pool.tile([128, n], BF16, name="sqy")
                    sqx = ws_pool.tile([128, n], BF16, name="sqx")
                    nc.scalar.activation(sqy[:], ay0[:], sq, bias=bm1[:])
                    nc.scalar.activation(sqx[:], ax0[:], sq, bias=bm1[:])
                    st[i] = (sqx, sqy, ev, n)

                zcol = {}
                for k, i in enumerate(sorted(direct_z)):
                    zcol[i] = NSU + 1 + k

                def stage_c(i):
                    sqx, sqy, ev, n = st.pop(i)
                    w = wz_pool.tile([128, n], BF16, name="w")
                    z = wz_pool.tile([128, n], BF16, name="z")
                    nc.vector.scalar_tensor_tensor(
                        w[:], sqy[:], 1.0, ev,
                        op0=mybir.AluOpType.subtract, op1=mybir.AluOpType.mult,
                        accum_out=partials[:, i : i + 1],
                    )
                    if i in direct_z:
                        nc.vector.scalar_tensor_tensor(
                            z[:], sqx[:], 0.0, w[:],
                            op0=mybir.AluOpType.add, op1=mybir.AluOpType.mult,
                            accum_out=partials[:, zcol[i] : zcol[i] + 1],
                        )
                    else:
                        zmul = (nc.gpsimd.tensor_mul if pool_z
                                else nc.vector.tensor_mul)
                        zmul(z[:], sqx[:], w[:])
                        zs[i] = (z, n)

                # PSUM range bookkeeping: first/last subunit touching each
                sub_n = []
                for i in range(NSU):
                    u, ci = subs[i]
                    sub_n.append(
                        6 * (plans[u][ci][1] - plans[u][ci][0]) * 6
                        if u >= 0 else W_runt
                    )
                r_first, r_last = {}, {}
                for i, n in enumerate(sub_n):
                    if i in direct_z:
                        continue
                    for c0 in range(0, n, 512):
                        r = c0 // 512
                        r_first.setdefault(r, i)
                        r_last[r] = i

                def stage_pe(i):
                    if i in direct_z:
                        return
                    z, n = zs.pop(i)
                    for c0 in range(0, n, 512):
                        cw = min(512, n - c0)
                        r = c0 // 512
                        nc.tensor.matmul(
                            zacc[:, c0 : c0 + cw], ident[:], z[:, c0 : c0 + cw],
                            start=(r_first[r] == i), stop=(r_last[r] == i),
                        )

                stage_a(0)
                stage_b(0)
                for i in range(1, NSU):
                    stage_a(i)
                    stage_b(i)
                    stage_c(i - 1)
                    stage_pe(i - 1)
                stage_c(NSU - 1)
                stage_pe(NSU - 1)
                nc.scalar.activation(
                    zjunk[:], zacc[:], mybir.ActivationFunctionType.Copy,
                    accum_out=partials[:, NSU : NSU + 1],
                )
                nc.sync.dma_start(out_h.ap(), partials[:])

            if niter == 1:
                emit_iter()
            else:
                with tc.For_i(0, niter, 1):
                    emit_iter()

    nc.compile()
    return nc


def _stage(mask, edge, idx):
    """Host-side slicing: maskb [B*KXb, 7, NG, 7] and edgeb [B*KXb, 6, NG, 6]
    f32 (dense; mask zero outside the image)."""
    B, X, Y = mask.shape
    sx, sy = SHIFTS[idx]
    KX = (X + DX) // DX
    KY = (Y + DX) // DX

    gy = np.arange(KY)
    y0 = 8 * gy[:, None] + np.arange(7)[None, :] - 4 * sy  # [KY, 7]
    g_ok = (y0 >= 0) & (y0 < Y)
    gsel = np.nonzero(g_ok.any(axis=1))[0]
    NG = len(gsel)
    ym = y0[gsel]
    yv = g_ok[gsel]
    kxs = np.arange(KX)
    x0 = 8 * kxs[:, None] + np.arange(7)[None, :] - 4 * sx  # [KX, 7]
    k_ok = (x0 >= 0) & (x0 < X)
    ksel = np.nonzero(k_ok.any(axis=1))[0]
    KXb = len(ksel)
    xm = x0[ksel]
    xv = k_ok[ksel]

    mc = mask[:, np.clip(xm.ravel(), 0, X - 1), :][:, :, np.clip(ym.ravel(), 0, Y - 1)]
    mc = mc.reshape(B, KXb * 7, NG * 7)
    vmask = (xv.ravel()[:, None] & yv.ravel()[None, :]).astype(mask.dtype)
    mc *= vmask
    maskb = mc.reshape(B * KXb, 7, NG, 7)

    xe = np.clip(xm[:, 0:6].ravel(), 0, X - 1)
    ye = np.clip(ym[:, 0:6].ravel(), 0, Y - 1)
    edgeb = edge[:, xe, :][:, :, ye].reshape(B * KXb, 6, NG, 6)

    norm = B * KX * KY
    return maskb, edgeb, NG, KXb, norm


def _stage_runt(maskb4, edgeb4, sel):
    """Gather leftover blocks' vb/vr/vc/e cells into [128, 4*W] (W padded)."""
    m4 = maskb4[sel]
    e4 = edgeb4[sel]
    vb = m4[:, 0:6, :, 0:6].ravel()
    vr = m4[:, 1:7, :, 0:6].ravel()
    vc = m4[:, 0:6, :, 1:7].ravel()
    ee = e4.ravel()
    n = vb.size
    W = -(-n // 128)
    out = np.zeros((4, 128 * W), dtype=maskb4.dtype)
    for i, a in enumerate((vb, vr, vc, ee)):
        out[i, :n] = a
    return np.ascontiguousarray(
        out.reshape(4, 128, W).transpose(1, 0, 2).reshape(128, 4 * W)
    ), W


def _run(mask, edge, loss_old, idx, trace=False, niter=1, pool_z=False):
    import ml_dtypes

    B, X, Y, _ = mask.shape
    assert B % N_CORES == 0
    m3 = np.ascontiguousarray(mask[..., idx], dtype=np.float32)
    e3 = np.ascontiguousarray(edge[..., 0], dtype=np.float32)
    maskb, edgeb, NG, KXb, norm = _stage(m3, e3, idx)

    NBtot = maskb.shape[0]
    assert NBtot % N_CORES == 0
    NBc = NBtot // N_CORES
    n_full = NBc // 128
    runt_sel0 = np.arange(n_full * 128, NBc)
    h = NG // 2

    maskb4 = maskb.astype(ml_dtypes.bfloat16)
    edgeb4 = edgeb.astype(ml_dtypes.bfloat16)
    plans = _chunks_for(NG, n_full)
    ident = np.eye(128, dtype=ml_dtypes.bfloat16)

    in_maps = []
    W_runt = 0
    for i in range(N_CORES):
        lo = i * NBc
        im = {"ident": ident}
        for u in range(n_full):
            r0 = lo + u * 128
            for ci, (g0, g1) in enumerate(plans[u]):
                im[f"m{u}_{ci}"] = np.ascontiguousarray(
                    maskb4[r0 : r0 + 128, :, g0:g1, :]
                ).reshape(128, -1)
            for s, (g0, g1) in enumerate([(0, h), (h, NG)]):
                im[f"e{u}_{s}"] = np.ascontiguousarray(
                    edgeb4[r0 : r0 + 128, :, g0:g1, :]
                ).reshape(128, -1)
        if len(runt_sel0):
            rb, W_runt = _stage_runt(maskb4, edgeb4, lo + runt_sel0)
            im["runtb"] = rb
        in_maps.append(im)

    nc = _build_program(n_full, NG, W_runt, niter=niter, pool_z=pool_z)
    res = run_bass_kernel_spmd(nc, in_maps, list(range(N_CORES)), trace=trace)
    # w columns are the first NSU; z columns (zacc total + direct) follow
    NSU = sum(len(p) for p in plans) + (1 if W_runt else 0)
    total = 0.0
    for i in range(N_CORES):
        o = np.asarray(res.results[i]["out"], np.float64)
        total += o[:, NSU:].sum() - o[:, :NSU].sum()
    out = np.float32(np.asarray(loss_old, dtype=np.float32) + total / norm)
    return np.asarray(out, dtype=np.float32), res


def kernel(resized_image, mask_combined, edge_map, loss_old, mask_index):
    mask = np.asarray(mask_combined, dtype=np.float32)
    edge = np.asarray(edge_map, dtype=np.float32)
    idx = int(np.asarray(mask_index))
    out, _ = _run(mask, edge, loss_old, idx)
    return out
